# revision 18
# baseline (speedup 1.0000x reference)
"""Multi-head self-attention Trainium2 kernel (B=4, S=2048, D=1024, H=16, dk=64).

Sharding (8 cores): data-parallel over batch (4) x tensor-parallel over head
groups (2).  Core c handles batch c//2 and heads [8*(c%2), 8*(c%2)+8), i.e.
feature columns [512*(c%2), 512*(c%2)+512) of Wq/Wk/Wv (column split) and the
matching rows of Wo (row split).  Each core emits a partial [2048, 1024]
output; the host sums the two partials per batch and adds bo.

Engine budget per core per rep (target ~270us): PE 273us (projections 82,
scores 55 row-tiled-concurrent, EV 109, out-proj 27), ACT ~220us (12/16 of
softmax exp tiles + Q/K psum evac with fused bias), DVE ~205us (4/16 exp
tiles via 2-instruction custom exp2, V/out evac, fast reciprocal), GPSIMD
~50us (denominator broadcast + normalize multiply).

Key structure vs a naive version:
  - x^T streamed once per rep as bf16; all weights loaded once outside the
    repeat loop (weights-resident steady state).
  - Q^T/K^T feature-major bf16 so scores K_h Q_h^T needs no transposes; the
    two heads of a pair sit on partition halves and their scores matmuls run
    CONCURRENTLY via PE row tiling (tile_position (0,0)/(64,0)).
  - 1/sqrt(dk) and log2(e) folded into Wq/bq: scores arrive in log2 domain,
    so softmax exp = 2^z: ACT does exp(z*ln2), DVE tiles use a custom
    exp2 pair (exponent-bit assembly + quadratic mantissa correction,
    max rel err 1.7e-3).
  - V stored [keys, 65] per head with a ones column: the EV matmul emits
    both EV^T and the softmax denominator in one accumulation.
  - QT/KT/V65 double-buffered so rep r+1's projections overlap rep r's
    attention (cross-rep software pipelining).
"""

import numpy as np
import ml_dtypes

import concourse.mybir as mybir
import concourse.tile as tile
from concourse import bacc
from concourse.bass_utils import run_bass_kernel_spmd

import concourse.dve_ops as _dve_ops
from concourse.dve_ops import DveOp as _DveOp, OPS as _DVE_OPS
from concourse.dve_spec import (
    C0 as _C0, C1 as _C1, C2 as _C2, Spec as _Spec, Src0 as _Src0,
    Src1 as _Src1, lower as _dve_lower, sq as _sq,
)
from concourse.dve_uop import DveOpSpec as _DveOpSpec

# --- custom DVE exp2 (2-instruction softmax-exp offload ACT -> DVE) -------
# I1: eb_i32 = round(z)*2^23 + bits(delta)  == fp32 bits of delta*2^round(z)
# I2: out = (b2*(f+alpha)^2)*eb + eb, f = z - round(z).  Combined:
# (1 + b2*(f+alpha)^2)*delta*2^round(z) ~= 2^z, max rel err 1.73e-3.
MAGIC = 12582912.0            # 1.5*2^23 magic round-to-nearest
P23 = 8388608.0               # 2^23
DELTA_BITS = 1056346816.0     # bits(0.48158836) = 0x3ef692c0 (low 6 bits 0)
ALPHA = 1.4751758607805647
B2 = 0.4950878485463483
LOG2E = 1.4426950408889634


def _exp2_eb_ref(in0, in1, s0, s1, imm2):
    r = np.round(in0.astype(np.float32))
    return (r * np.float32(imm2) + np.float32(s1)).astype(np.float32)


def _exp2_mul_ref(in0, in1, s0, s1, imm2):
    z = in0.astype(np.float32)
    f = (z - np.round(z)).astype(np.float32)
    g = (f + np.float32(s1)).astype(np.float32)
    eb = np.asarray(in1, np.float32)
    return (np.float32(imm2) * g * g) * eb + eb


def _register_exp2_ops():
    existing = {o.name: o for o in _DVE_OPS}
    if "EXP2_EB_ANT" in existing:
        return existing["EXP2_EB_ANT"], existing["EXP2_MUL_ANT"]
    eb_spec = _Spec(body=((_Src0 + _C0) - _C0) * _C2 + _C1, reference=_exp2_eb_ref)
    mul_g = (_Src0 - ((_Src0 + _C0) - _C0)) + _C1
    mul_spec = _Spec(body=_sq(mul_g) * _C2 * _Src1 + _Src1,
                     reference=_exp2_mul_ref)
    made = []
    for name, spec, rd1 in (("EXP2_EB_ANT", eb_spec, False),
                            ("EXP2_MUL_ANT", mul_spec, True)):
        row = _dve_ops._CUSTOM_DVE_ROW_BASE + len(_DVE_OPS)
        assert row < 0x20
        _dve_ops._SUB_OPCODE_FOR_NAME[name] = row
        sha = _DveOpSpec(name=name, opcode=row,
                         uops=_dve_lower(spec, ver="v3"), rd1_en=rd1).sha("v3")
        op = _DveOp(name, spec, subdim=False, uops_sha={"v3": sha})
        _DVE_OPS.append(op)
        _dve_ops.CUSTOM_DVE_SPECS[name] = spec
        made.append(op)
    return made[0], made[1]


EXP2_EB, EXP2_MUL = _register_exp2_ops()

F32 = mybir.dt.float32
I32 = mybir.dt.int32
BF16 = mybir.dt.bfloat16

P = 128
D = 1024          # model dim
S = 2048          # sequence length
FH = 512          # local feature width (8 heads x 64)
H_LOC = 8         # heads per core
DK = 64           # head dim
N_DT = D // P     # 8 d-tiles
N_FT = FH // P    # 4 local feature tiles
N_ST = S // P     # 16 sequence tiles
N_SC = S // 512   # 4 sequence chunks of 512
QC = 512          # query chunk
LN2 = 0.6931471805599453
DVE_KT = frozenset(())   # exp tiles offloaded ACT -> DVE


def _load_consts(nc, tc, pools, wq, bqc, wk, bkc, wv, bv, wo):
    consts = pools["consts"]
    bq_sb = consts.tile([P, N_FT], F32, name="bq")
    nc.sync.dma_start(out=bq_sb, in_=bqc[:, :].rearrange("(ft p) o -> p (ft o)", p=P))
    bk_sb = consts.tile([P, N_FT], F32, name="bk")
    nc.sync.dma_start(out=bk_sb, in_=bkc[:, :].rearrange("(ft p) o -> p (ft o)", p=P))
    bv_row = consts.tile([1, FH], BF16, name="bvr")
    nc.sync.dma_start(out=bv_row, in_=bv[:, :])
    bvb = consts.tile([P, FH], BF16, name="bvb")
    nc.gpsimd.partition_broadcast(bvb, bv_row)

    ws = {}
    for nm, wd in (("wq", wq), ("wk", wk), ("wv", wv)):
        w_sb = consts.tile([P, N_DT, FH], BF16, name=nm)
        nc.sync.dma_start(
            out=w_sb, in_=wd[:, :].rearrange("(dt p) f -> p dt f", p=P))
        ws[nm] = w_sb
    wo_sb = consts.tile([P, N_FT, D], BF16, name="wo")
    nc.sync.dma_start(
        out=wo_sb, in_=wo[:, :].rearrange("(ft p) e -> p ft e", p=P))
    return dict(bq=bq_sb, bk=bk_sb, bvb=bvb, wo=wo_sb, **ws)


def _emit(nc, tc, pools, cs, xT, out, taps=None):
    Exp = mybir.ActivationFunctionType.Exp
    Id = mybir.ActivationFunctionType.Identity
    persist, xt_pool = pools["persist"], pools["xt"]
    psA, psSC, psEV = pools["psA"], pools["psSC"], pools["psEV"]
    e_pool, eb_pool, r_pool, o_pool = (
        pools["e"], pools["eb"], pools["r"], pools["o"])

    QT = persist.tile([P, N_FT, S], BF16, tag="QT")
    KT = persist.tile([P, N_FT, S], BF16, tag="KT")
    V65 = persist.tile([P, N_ST, H_LOC, DK + 1], BF16, tag="V65")
    nc.vector.memset(V65[:, :, :, DK:DK + 1], 1.0)

    # ---------------- Phase A: projections ----------------
    xts = []
    for dt in range(N_DT):
        xt = xt_pool.tile([P, S], BF16, tag=f"xt{dt}", name=f"xt{dt}")
        nc.sync.dma_start(out=xt, in_=xT[dt * P:(dt + 1) * P, :])
        xts.append(xt)

    # Q^T and K^T, feature-major: psum[f, s] = sum_d W[d, f] x^T[d, s];
    # bias added on ACT during psum evacuation (per-partition bias AP).
    for w_sb, bias_sb, dest in ((cs["wq"], cs["bq"], QT),
                                (cs["wk"], cs["bk"], KT)):
        for ft in range(N_FT):
            for sc in range(N_SC):
                ps = psA.tile([P, QC], F32, tag="psA", name="psqk")
                for dt in range(N_DT):
                    nc.tensor.matmul(
                        ps,
                        (w_sb[:, dt, ft * P:(ft + 1) * P]),
                        (xts[dt][:, sc * QC:(sc + 1) * QC]),
                        start=(dt == 0), stop=(dt == N_DT - 1))
                nc.scalar.activation(
                    out=dest[:, ft, sc * QC:(sc + 1) * QC], in_=ps,
                    func=Id, bias=bias_sb[:, ft:ft + 1], scale=1.0)

    # V natural: psum[s, f] = sum_d x^T[d, s] W[d, f]; bias via DVE add.
    for st in range(N_ST):
        ps = psA.tile([P, FH], F32, tag="psA", name="psv")
        for dt in range(N_DT):
            nc.tensor.matmul(
                ps,
                (xts[dt][:, st * P:(st + 1) * P]),
                (cs["wv"][:, dt, :]),
                start=(dt == 0), stop=(dt == N_DT - 1))
        nc.vector.tensor_add(
            out=V65[:, st, :, 0:DK],
            in0=ps[:, :].rearrange("p (h d) -> p h d", h=H_LOC),
            in1=cs["bvb"][:, :].rearrange("p (h d) -> p h d", h=H_LOC))

    # ---------------- Phase B: attention ----------------
    AO = pools["ao"].tile([P, N_FT, S], BF16, tag="AO")
    for t in range(N_FT):
        for qc in range(N_SC):
            ev = [psEV.tile([DK + 1, QC], F32, tag="ev",
                            name=f"ev{h2}") for h2 in range(2)]
            for kt in range(N_ST):
                # scoresT[j, i] for the head pair (2t, 2t+1): rows 0-63 of
                # KT/QT tile t = head 2t, rows 64-127 = head 2t+1; the two
                # matmuls run concurrently via PE row tiling.
                ps = psSC.tile([P, 2 * QC], F32, tag="sc", name="scps")
                for h2 in range(2):
                    lo = h2 * DK
                    nc.tensor.matmul(
                        ps[:, h2 * QC:(h2 + 1) * QC],
                        (KT[lo:lo + DK, t, kt * P:(kt + 1) * P]),
                        (QT[lo:lo + DK, t, qc * QC:(qc + 1) * QC]),
                        start=True, stop=True,
                        skip_group_check=True)
                e = e_pool.tile([P, 2 * QC], BF16, tag="e", name="esb")
                if kt in DVE_KT:
                    ebt = eb_pool.tile([P, 2 * QC], I32, tag="eb", name="ebt")
                    nc.vector._custom_dve(
                        EXP2_EB, out=ebt, in0=ps,
                        s0=MAGIC, s1=DELTA_BITS, imm2=P23)
                    nc.vector._custom_dve(
                        EXP2_MUL, out=e, in0=ps, in1=ebt.bitcast(F32),
                        s0=MAGIC, s1=ALPHA, imm2=B2)
                else:
                    nc.scalar.activation(out=e, in_=ps, func=Exp, scale=LN2)
                for h2 in range(2):
                    nc.tensor.matmul(
                        ev[h2],
                        V65[:, kt, 2 * t + h2, :],
                        e[:, h2 * QC:(h2 + 1) * QC],
                        start=(kt == 0), stop=(kt == N_ST - 1),
                        skip_group_check=True)
            for h2 in range(2):
                # normalize: AO^T[f, i] = EV^T[f, i] / d[i]
                dd = r_pool.tile([1, QC], F32, tag="dd", name="dd")
                nc.vector.tensor_copy(out=dd, in_=ev[h2][DK:DK + 1, :])
                r1 = r_pool.tile([1, QC], F32, tag="r1", name="r1")
                nc.vector.reciprocal_approx_fast(out=r1, in_=dd)
                rb = r_pool.tile([DK, QC], F32, tag="rb", name="rb")
                nc.gpsimd.partition_broadcast(rb, r1)
                nc.vector.tensor_mul(
                    out=AO[h2 * DK:(h2 + 1) * DK, t,
                           qc * QC:(qc + 1) * QC],
                    in0=ev[h2][0:DK, :], in1=rb)
                if taps is not None and t == 0 and qc == 0 and h2 == 0:
                    evc = r_pool.tile([DK + 1, QC], F32, tag="evc", name="evc")
                    nc.vector.tensor_copy(out=evc, in_=ev[0][:, :])
                    nc.sync.dma_start(out=taps["ev0"][:, :], in_=evc)
                    nc.sync.dma_start(out=taps["r1"][:, :], in_=r1)
                    nc.sync.dma_start(out=taps["rb"][:, :], in_=rb)

    if taps is not None:
        nc.sync.dma_start(out=taps["qt"][:, :],
                          in_=QT[:, :, :].rearrange("p a b -> p (a b)"))
        nc.sync.dma_start(out=taps["kt"][:, :],
                          in_=KT[:, :, :].rearrange("p a b -> p (a b)"))
        nc.sync.dma_start(out=taps["v"][:, :],
                          in_=V65[:, :, :, :].rearrange("p a b c -> p (a b c)"))
        nc.sync.dma_start(out=taps["ao"][:, :],
                          in_=AO[:, :, :].rearrange("p a b -> p (a b)"))

    # ---------------- Phase C: output projection ----------------
    for st in range(N_ST):
        for ec in range(D // QC):
            ps = psA.tile([P, QC], F32, tag="psA", name="pso")
            for ft in range(N_FT):
                nc.tensor.matmul(
                    ps,
                    (AO[:, ft, st * P:(st + 1) * P]),
                    (cs["wo"][:, ft, ec * QC:(ec + 1) * QC]),
                    start=(ft == 0), stop=(ft == N_FT - 1))
            ob = o_pool.tile([P, QC], F32, tag="ob", name="ob")
            nc.vector.tensor_copy(out=ob, in_=ps)
            nc.sync.dma_start(
                out=out[st * P:(st + 1) * P, ec * QC:(ec + 1) * QC],
                in_=ob)


def build_nc(debug=False, repeat=1):
    nc = bacc.Bacc("TRN2", debug=debug)
    xT = nc.declare_dram_parameter("xT", [D, S], BF16, isOutput=False)
    wq = nc.declare_dram_parameter("wq", [D, FH], BF16, isOutput=False)
    bqc = nc.declare_dram_parameter("bqc", [FH, 1], F32, isOutput=False)
    wk = nc.declare_dram_parameter("wk", [D, FH], BF16, isOutput=False)
    bkc = nc.declare_dram_parameter("bkc", [FH, 1], F32, isOutput=False)
    wv = nc.declare_dram_parameter("wv", [D, FH], BF16, isOutput=False)
    bv = nc.declare_dram_parameter("bv", [1, FH], BF16, isOutput=False)
    wo = nc.declare_dram_parameter("wo", [FH, D], BF16, isOutput=False)
    out = nc.declare_dram_parameter("out", [S, D], F32, isOutput=True)
    with tile.TileContext(nc) as tc:
        with tc.tile_pool(name="consts", bufs=1) as consts, \
             tc.tile_pool(name="persist", bufs=2) as persist, \
             tc.tile_pool(name="ao", bufs=1) as ao, \
             tc.tile_pool(name="xt", bufs=1) as xt, \
             tc.tile_pool(name="e", bufs=4) as e_pool, \
             tc.tile_pool(name="eb", bufs=1) as eb_pool, \
             tc.tile_pool(name="r", bufs=2) as r_pool, \
             tc.tile_pool(name="o", bufs=2) as o_pool, \
             tc.tile_pool(name="psA", bufs=1, space="PSUM") as psA, \
             tc.tile_pool(name="psSC", bufs=2, space="PSUM") as psSC, \
             tc.tile_pool(name="psEV", bufs=3, space="PSUM") as psEV:
            pools = dict(consts=consts, persist=persist, ao=ao, xt=xt,
                         e=e_pool, eb=eb_pool, r=r_pool, o=o_pool,
                         psA=psA, psSC=psSC, psEV=psEV)
            cs = _load_consts(nc, tc, pools, wq[:, :], bqc[:, :], wk[:, :],
                              bkc[:, :], wv[:, :], bv[:, :], wo[:, :])
            for _rep in range(repeat):
                _emit(nc, tc, pools, cs, xT[:, :], out[:, :])
    nc.compile()
    return nc


def make_in_maps(x, Wq, bq, Wk, bk, Wv, bv, Wo):
    bf = ml_dtypes.bfloat16
    sc = 0.125 * LOG2E
    in_maps = []
    for c in range(8):
        b, hg = divmod(c, 2)
        F = slice(FH * hg, FH * (hg + 1))
        in_maps.append({
            "xT": np.ascontiguousarray(x[b].T).astype(bf),
            "wq": (np.ascontiguousarray(Wq[:, F]) * sc).astype(bf),
            "bqc": (bq[F] * sc).astype(np.float32).reshape(FH, 1),
            "wk": np.ascontiguousarray(Wk[:, F]).astype(bf),
            "bkc": np.ascontiguousarray(bk[F]).astype(np.float32).reshape(FH, 1),
            "wv": np.ascontiguousarray(Wv[:, F]).astype(bf),
            "bv": np.ascontiguousarray(bv[F]).astype(bf).reshape(1, FH),
            "wo": np.ascontiguousarray(Wo[F, :]).astype(bf),
        })
    return in_maps


_NC_CACHE = None


def _get_nc():
    global _NC_CACHE
    if _NC_CACHE is None:
        _NC_CACHE = build_nc()
    return _NC_CACHE


def kernel(x, Wq, bq, Wk, bk, Wv, bv, Wo, bo, _trace=False):
    x = np.asarray(x, np.float32)
    args = [np.asarray(a, np.float32) for a in (Wq, bq, Wk, bk, Wv, bv, Wo)]
    bo = np.asarray(bo, np.float32)
    nc = _get_nc()
    in_maps = make_in_maps(x, *args)
    res = run_bass_kernel_spmd(nc, in_maps, list(range(8)), trace=_trace)
    out = np.empty((4, S, D), np.float32)
    for b in range(4):
        out[b] = res.results[2 * b]["out"] + res.results[2 * b + 1]["out"] + bo
    if _trace:
        return out, res
    return out


# revision 21
# speedup vs baseline: 1.1912x; 1.1912x over previous
"""Multi-head self-attention Trainium2 kernel (B=4, S=2048, D=1024, H=16, dk=64).

Sharding (8 cores): data-parallel over batch (4) x tensor-parallel over head
groups (2).  Core c handles batch c//2 and heads [8*(c%2), 8*(c%2)+8), i.e.
feature columns [512*(c%2), 512*(c%2)+512) of Wq/Wk/Wv (column split) and the
matching rows of Wo (row split).  Each core emits a partial [2048, 1024]
output; the host sums the two partials per batch and adds bo.

Engine budget per core per rep (target ~270us): PE 273us (projections 82,
scores 55 row-tiled-concurrent, EV 109, out-proj 27), ACT ~220us (12/16 of
softmax exp tiles + Q/K psum evac with fused bias), DVE ~205us (4/16 exp
tiles via 2-instruction custom exp2, V/out evac, fast reciprocal), GPSIMD
~50us (denominator broadcast + normalize multiply).

Key structure vs a naive version:
  - x^T streamed once per rep as bf16; all weights loaded once outside the
    repeat loop (weights-resident steady state).
  - Q^T/K^T feature-major bf16 so scores K_h Q_h^T needs no transposes; the
    two heads of a pair sit on partition halves and their scores matmuls run
    CONCURRENTLY via PE row tiling (tile_position (0,0)/(64,0)).
  - 1/sqrt(dk) and log2(e) folded into Wq/bq: scores arrive in log2 domain,
    so softmax exp = 2^z: ACT does exp(z*ln2), DVE tiles use a custom
    exp2 pair (exponent-bit assembly + quadratic mantissa correction,
    max rel err 1.7e-3).
  - V stored [keys, 65] per head with a ones column: the EV matmul emits
    both EV^T and the softmax denominator in one accumulation.
  - QT/KT/V65 double-buffered so rep r+1's projections overlap rep r's
    attention (cross-rep software pipelining).
"""

import numpy as np
import ml_dtypes

import concourse.mybir as mybir
import concourse.tile as tile
from concourse import bacc
from concourse.bass_utils import run_bass_kernel_spmd

import concourse.dve_ops as _dve_ops
from concourse.dve_ops import DveOp as _DveOp, OPS as _DVE_OPS
from concourse.dve_spec import (
    C0 as _C0, C1 as _C1, C2 as _C2, Spec as _Spec, Src0 as _Src0,
    Src1 as _Src1, lower as _dve_lower, sq as _sq,
)
from concourse.dve_uop import DveOpSpec as _DveOpSpec

# --- custom DVE exp2 (2-instruction softmax-exp offload ACT -> DVE) -------
# I1: eb_i32 = round(z)*2^23 + bits(delta)  == fp32 bits of delta*2^round(z)
# I2: out = (b2*(f+alpha)^2)*eb + eb, f = z - round(z).  Combined:
# (1 + b2*(f+alpha)^2)*delta*2^round(z) ~= 2^z, max rel err 1.73e-3.
MAGIC = 12582912.0            # 1.5*2^23 magic round-to-nearest
P23 = 8388608.0               # 2^23
DELTA_BITS = 1056346816.0     # bits(0.48158836) = 0x3ef692c0 (low 6 bits 0)
ALPHA = 1.4751758607805647
B2 = 0.4950878485463483
LOG2E = 1.4426950408889634


def _exp2_eb_ref(in0, in1, s0, s1, imm2):
    r = np.round(in0.astype(np.float32))
    return (r * np.float32(imm2) + np.float32(s1)).astype(np.float32)


def _exp2_mul_ref(in0, in1, s0, s1, imm2):
    z = in0.astype(np.float32)
    f = (z - np.round(z)).astype(np.float32)
    g = (f + np.float32(s1)).astype(np.float32)
    eb = np.asarray(in1, np.float32)
    return (np.float32(imm2) * g * g) * eb + eb


def _register_exp2_ops():
    existing = {o.name: o for o in _DVE_OPS}
    if "EXP2_EB_ANT" in existing:
        return existing["EXP2_EB_ANT"], existing["EXP2_MUL_ANT"]
    eb_spec = _Spec(body=((_Src0 + _C0) - _C0) * _C2 + _C1, reference=_exp2_eb_ref)
    mul_g = (_Src0 - ((_Src0 + _C0) - _C0)) + _C1
    mul_spec = _Spec(body=_sq(mul_g) * _C2 * _Src1 + _Src1,
                     reference=_exp2_mul_ref)
    made = []
    for name, spec, rd1 in (("EXP2_EB_ANT", eb_spec, False),
                            ("EXP2_MUL_ANT", mul_spec, True)):
        row = _dve_ops._CUSTOM_DVE_ROW_BASE + len(_DVE_OPS)
        assert row < 0x20
        _dve_ops._SUB_OPCODE_FOR_NAME[name] = row
        sha = _DveOpSpec(name=name, opcode=row,
                         uops=_dve_lower(spec, ver="v3"), rd1_en=rd1).sha("v3")
        op = _DveOp(name, spec, subdim=False, uops_sha={"v3": sha})
        _DVE_OPS.append(op)
        _dve_ops.CUSTOM_DVE_SPECS[name] = spec
        made.append(op)
    return made[0], made[1]


EXP2_EB, EXP2_MUL = _register_exp2_ops()

F32 = mybir.dt.float32
F32R = mybir.dt.float32r
I32 = mybir.dt.int32
BF16 = mybir.dt.bfloat16

P = 128
D = 1024          # model dim
S = 2048          # sequence length
FH = 512          # local feature width (8 heads x 64)
H_LOC = 8         # heads per core
DK = 64           # head dim
N_DT = D // P     # 8 d-tiles
N_FT = FH // P    # 4 local feature tiles
N_ST = S // P     # 16 sequence tiles
N_SC = S // 512   # 4 sequence chunks of 512
QC = 512          # query chunk
LN2 = 0.6931471805599453
SCHB = 1064866823.0   # debiased Schraudolph bias
DVE_KT = frozenset((3, 7, 11, 15))   # exp tiles offloaded ACT -> DVE


def _load_consts(nc, tc, pools, wq, bqc, wk, bkc, wv, bv, wo):
    consts = pools["consts"]
    bq_sb = consts.tile([P, N_FT], F32, name="bq")
    nc.sync.dma_start(out=bq_sb, in_=bqc[:, :].rearrange("(ft p) o -> p (ft o)", p=P))
    bk_sb = consts.tile([P, N_FT], F32, name="bk")
    nc.sync.dma_start(out=bk_sb, in_=bkc[:, :].rearrange("(ft p) o -> p (ft o)", p=P))
    bv_row = consts.tile([1, FH], BF16, name="bvr")
    nc.sync.dma_start(out=bv_row, in_=bv[:, :])
    bvb = consts.tile([P, FH], BF16, name="bvb")
    nc.gpsimd.partition_broadcast(bvb, bv_row)

    ws = {}
    for nm, wd in (("wq", wq), ("wk", wk), ("wv", wv)):
        w_sb = consts.tile([P, N_DT, FH], BF16, name=nm)
        nc.sync.dma_start(
            out=w_sb, in_=wd[:, :].rearrange("(dt p) f -> p dt f", p=P))
        ws[nm] = w_sb
    wo_sb = consts.tile([P, N_FT, D], BF16, name="wo")
    nc.sync.dma_start(
        out=wo_sb, in_=wo[:, :].rearrange("(ft p) e -> p ft e", p=P))
    return dict(bq=bq_sb, bk=bk_sb, bvb=bvb, wo=wo_sb, **ws)


def _emit(nc, tc, pools, cs, xT, out, taps=None):
    Exp = mybir.ActivationFunctionType.Exp
    Id = mybir.ActivationFunctionType.Identity
    persist, xt_pool = pools["persist"], pools["xt"]
    psA, psSC, psEV = pools["psA"], pools["psSC"], pools["psEV"]
    e_pool, eb_pool, r_pool, o_pool = (
        pools["e"], pools["eb"], pools["r"], pools["o"])

    QT = persist.tile([P, N_FT, S], BF16, tag="QT")
    KT = persist.tile([P, N_FT, S], BF16, tag="KT")
    V65 = persist.tile([P, N_ST, H_LOC, DK + 1], BF16, tag="V65")
    nc.vector.memset(V65[:, :, :, DK:DK + 1], 1.0)

    # ---------------- Phase A: projections ----------------
    xts = []
    for dt in range(N_DT):
        xt = xt_pool.tile([P, S], BF16, tag=f"xt{dt}", name=f"xt{dt}")
        nc.sync.dma_start(out=xt, in_=xT[dt * P:(dt + 1) * P, :])
        xts.append(xt)

    # Q^T and K^T, feature-major: psum[f, s] = sum_d W[d, f] x^T[d, s];
    # bias added on ACT during psum evacuation (per-partition bias AP).
    for w_sb, bias_sb, dest in ((cs["wq"], cs["bq"], QT),
                                (cs["wk"], cs["bk"], KT)):
        for ft in range(N_FT):
            for sc in range(N_SC):
                ps = psA.tile([P, QC], F32, tag="psA", name="psqk")
                for dt in range(N_DT):
                    nc.tensor.matmul(
                        ps,
                        (w_sb[:, dt, ft * P:(ft + 1) * P]),
                        (xts[dt][:, sc * QC:(sc + 1) * QC]),
                        start=(dt == 0), stop=(dt == N_DT - 1))
                nc.scalar.activation(
                    out=dest[:, ft, sc * QC:(sc + 1) * QC], in_=ps,
                    func=Id, bias=bias_sb[:, ft:ft + 1], scale=1.0)

    # V natural: psum[s, f] = sum_d x^T[d, s] W[d, f]; bias via DVE add.
    for st in range(N_ST):
        ps = psA.tile([P, FH], F32, tag="psA", name="psv")
        for dt in range(N_DT):
            nc.tensor.matmul(
                ps,
                (xts[dt][:, st * P:(st + 1) * P]),
                (cs["wv"][:, dt, :]),
                start=(dt == 0), stop=(dt == N_DT - 1))
        nc.vector.tensor_add(
            out=V65[:, st, :, 0:DK],
            in0=ps[:, :].rearrange("p (h d) -> p h d", h=H_LOC),
            in1=cs["bvb"][:, :].rearrange("p (h d) -> p h d", h=H_LOC))

    # ---------------- Phase B: attention ----------------
    AO = pools["ao"].tile([P, N_FT, S], BF16, tag="AO")
    for t in range(N_FT):
        for qc in range(N_SC):
            ev = [psEV.tile([DK + 1, QC], F32, tag="ev",
                            name=f"ev{h2}") for h2 in range(2)]
            for kt in range(N_ST):
                # scoresT[j, i] for the head pair (2t, 2t+1): rows 0-63 of
                # KT/QT tile t = head 2t, rows 64-127 = head 2t+1; the two
                # matmuls run concurrently via PE row tiling.
                ps = psSC.tile([P, 2 * QC], F32, tag="sc", name="scps")
                for h2 in range(2):
                    lo = h2 * DK
                    nc.tensor.matmul(
                        ps[:, h2 * QC:(h2 + 1) * QC],
                        (KT[lo:lo + DK, t, kt * P:(kt + 1) * P]),
                        (QT[lo:lo + DK, t, qc * QC:(qc + 1) * QC]),
                        start=True, stop=True,
                        skip_group_check=True)
                e = e_pool.tile([P, 2 * QC], BF16, tag="e", name="esb")
                if kt in DVE_KT:
                    # Schraudolph exp2 on DVE: int32 convert of z*2^23 + B
                    # yields fp32 bits of ~2^z; then convert to bf16.
                    edv = eb_pool.tile([P, 2 * QC], I32, tag="eb", name="edv")
                    nc.vector.tensor_scalar(
                        out=edv, in0=ps, scalar1=P23, scalar2=SCHB,
                        op0=mybir.AluOpType.mult, op1=mybir.AluOpType.add)
                    nc.vector.tensor_copy(out=e, in_=edv.bitcast(F32))
                else:
                    nc.scalar.activation(out=e, in_=ps, func=Exp, scale=LN2)
                for h2 in range(2):
                    nc.tensor.matmul(
                        ev[h2],
                        V65[:, kt, 2 * t + h2, :],
                        e[:, h2 * QC:(h2 + 1) * QC],
                        start=(kt == 0), stop=(kt == N_ST - 1),
                        skip_group_check=True)
            for h2 in range(2):
                # normalize: AO^T[f, i] = EV^T[f, i] / d[i]
                dd = r_pool.tile([1, QC], F32, tag="dd", name="dd")
                nc.vector.tensor_copy(out=dd, in_=ev[h2][DK:DK + 1, :])
                r1 = r_pool.tile([1, QC], F32, tag="r1", name="r1")
                nc.vector.reciprocal_approx_fast(out=r1, in_=dd)
                rb = r_pool.tile([DK, QC], F32, tag="rb", name="rb")
                nc.gpsimd.partition_broadcast(rb, r1)
                nc.vector.tensor_mul(
                    out=AO[h2 * DK:(h2 + 1) * DK, t,
                           qc * QC:(qc + 1) * QC],
                    in0=ev[h2][0:DK, :], in1=rb)
                if taps is not None and t == 0 and qc == 0 and h2 == 0:
                    evc = r_pool.tile([DK + 1, QC], F32, tag="evc", name="evc")
                    nc.vector.tensor_copy(out=evc, in_=ev[0][:, :])
                    nc.sync.dma_start(out=taps["ev0"][:, :], in_=evc)
                    nc.sync.dma_start(out=taps["r1"][:, :], in_=r1)
                    nc.sync.dma_start(out=taps["rb"][:, :], in_=rb)

    if taps is not None:
        nc.sync.dma_start(out=taps["qt"][:, :],
                          in_=QT[:, :, :].rearrange("p a b -> p (a b)"))
        nc.sync.dma_start(out=taps["kt"][:, :],
                          in_=KT[:, :, :].rearrange("p a b -> p (a b)"))
        nc.sync.dma_start(out=taps["v"][:, :],
                          in_=V65[:, :, :, :].rearrange("p a b c -> p (a b c)"))
        nc.sync.dma_start(out=taps["ao"][:, :],
                          in_=AO[:, :, :].rearrange("p a b -> p (a b)"))

    # ---------------- Phase C: output projection ----------------
    for st in range(N_ST):
        for ec in range(D // QC):
            ps = psA.tile([P, QC], F32, tag="psA", name="pso")
            for ft in range(N_FT):
                nc.tensor.matmul(
                    ps,
                    (AO[:, ft, st * P:(st + 1) * P]),
                    (cs["wo"][:, ft, ec * QC:(ec + 1) * QC]),
                    start=(ft == 0), stop=(ft == N_FT - 1))
            ob = o_pool.tile([P, QC], F32, tag="ob", name="ob")
            nc.vector.tensor_copy(out=ob, in_=ps)
            nc.sync.dma_start(
                out=out[st * P:(st + 1) * P, ec * QC:(ec + 1) * QC],
                in_=ob)


def build_nc(debug=False, repeat=1):
    nc = bacc.Bacc("TRN2", debug=debug)
    xT = nc.declare_dram_parameter("xT", [D, S], BF16, isOutput=False)
    wq = nc.declare_dram_parameter("wq", [D, FH], BF16, isOutput=False)
    bqc = nc.declare_dram_parameter("bqc", [FH, 1], F32, isOutput=False)
    wk = nc.declare_dram_parameter("wk", [D, FH], BF16, isOutput=False)
    bkc = nc.declare_dram_parameter("bkc", [FH, 1], F32, isOutput=False)
    wv = nc.declare_dram_parameter("wv", [D, FH], BF16, isOutput=False)
    bv = nc.declare_dram_parameter("bv", [1, FH], BF16, isOutput=False)
    wo = nc.declare_dram_parameter("wo", [FH, D], BF16, isOutput=False)
    out = nc.declare_dram_parameter("out", [S, D], F32, isOutput=True)
    with tile.TileContext(nc) as tc:
        with tc.tile_pool(name="consts", bufs=1) as consts, \
             tc.tile_pool(name="persist", bufs=2) as persist, \
             tc.tile_pool(name="ao", bufs=1) as ao, \
             tc.tile_pool(name="xt", bufs=1) as xt, \
             tc.tile_pool(name="e", bufs=3) as e_pool, \
             tc.tile_pool(name="eb", bufs=1) as eb_pool, \
             tc.tile_pool(name="r", bufs=2) as r_pool, \
             tc.tile_pool(name="o", bufs=2) as o_pool, \
             tc.tile_pool(name="psA", bufs=2, space="PSUM") as psA, \
             tc.tile_pool(name="psSC", bufs=2, space="PSUM") as psSC, \
             tc.tile_pool(name="psEV", bufs=2, space="PSUM") as psEV:
            pools = dict(consts=consts, persist=persist, ao=ao, xt=xt,
                         e=e_pool, eb=eb_pool, r=r_pool, o=o_pool,
                         psA=psA, psSC=psSC, psEV=psEV)
            cs = _load_consts(nc, tc, pools, wq[:, :], bqc[:, :], wk[:, :],
                              bkc[:, :], wv[:, :], bv[:, :], wo[:, :])
            for _rep in range(repeat):
                _emit(nc, tc, pools, cs, xT[:, :], out[:, :])
    nc.compile()
    return nc


def make_in_maps(x, Wq, bq, Wk, bk, Wv, bv, Wo):
    bf = ml_dtypes.bfloat16
    sc = 0.125 * LOG2E
    in_maps = []
    for c in range(8):
        b, hg = divmod(c, 2)
        F = slice(FH * hg, FH * (hg + 1))
        in_maps.append({
            "xT": np.ascontiguousarray(x[b].T).astype(bf),
            "wq": (np.ascontiguousarray(Wq[:, F]) * sc).astype(bf),
            "bqc": (bq[F] * sc).astype(np.float32).reshape(FH, 1),
            "wk": np.ascontiguousarray(Wk[:, F]).astype(bf),
            "bkc": np.ascontiguousarray(bk[F]).astype(np.float32).reshape(FH, 1),
            "wv": np.ascontiguousarray(Wv[:, F]).astype(bf),
            "bv": np.ascontiguousarray(bv[F]).astype(bf).reshape(1, FH),
            "wo": np.ascontiguousarray(Wo[F, :]).astype(bf),
        })
    return in_maps


_NC_CACHE = None


def _get_nc():
    global _NC_CACHE
    if _NC_CACHE is None:
        _NC_CACHE = build_nc()
    return _NC_CACHE


def kernel(x, Wq, bq, Wk, bk, Wv, bv, Wo, bo, _trace=False):
    x = np.asarray(x, np.float32)
    args = [np.asarray(a, np.float32) for a in (Wq, bq, Wk, bk, Wv, bv, Wo)]
    bo = np.asarray(bo, np.float32)
    nc = _get_nc()
    in_maps = make_in_maps(x, *args)
    res = run_bass_kernel_spmd(nc, in_maps, list(range(8)), trace=_trace)
    out = np.empty((4, S, D), np.float32)
    for b in range(4):
        out[b] = res.results[2 * b]["out"] + res.results[2 * b + 1]["out"] + bo
    if _trace:
        return out, res
    return out
